# revision 1
# baseline (speedup 1.0000x reference)
"""Trainium2 Bass kernel for nn_C4ByteNibbleVM.

The reference "soft VM" computes, per 32-bit word (4 bytes, one-hot f32
encoded), out = onehot(((a + b) mod 2^32) ^ a) bytewise with a ripple
carry.  With exact one-hot inputs every softmax in the reference is
saturated (logit gaps >= 20), so the reference output equals the exact
integer result to ~1e-7.  The kernel therefore:
  1. extracts byte indices from the one-hot inputs (dot with iota),
  2. does the 4-byte ripple-carry add + xor in integer arithmetic,
  3. re-expands to one-hot via is_equal against an iota row.
Data parallel over the word dimension: 8192 words per core x 8 cores.
"""

import numpy as np
import ml_dtypes

import concourse.bacc as bacc
import concourse.mybir as mybir
from concourse.tile import TileContext
from concourse import bass_utils

B = 65536
NCORES = 8
BLOC = B // NCORES          # words per core
W = 8                       # 128-row chunks per iteration (1024 words)
ROWS_PER_ITER = 128 * W
NITER = BLOC // (128 * W)

F32 = mybir.dt.float32
BF16 = mybir.dt.bfloat16
I32 = mybir.dt.int32
AX = mybir.AxisListType
OP = mybir.AluOpType


def build_kernel(n_words=BLOC, w=W, reps=1, mode="full", bufs_ld=2, bufs_out=2, store="gpsimd", batch_g=0, bufs_sm=2, bufs_mul=3, bufs_idx=2):
    """Build the per-core Bass module. n_words must divide into 128*w tiles.

    reps>1 repeats the whole computation (same I/O) inside one NEFF so
    steady-state per-rep time can be measured by differencing wall times.
    mode: "full" | "dma" (I/O only) | "compute" (DVE only, minimal DMA).
    """
    rows_per_iter = 128 * w
    n_iter = n_words // rows_per_iter
    fd = 1024 * w  # free dim of one iteration tile

    nc = bacc.Bacc("TRN2", target_bir_lowering=False, debug=False)
    a_d = nc.dram_tensor("a", [n_words, 1024], F32, kind="ExternalInput")
    b_d = nc.dram_tensor("b", [n_words, 1024], F32, kind="ExternalInput")
    iota_d = nc.dram_tensor("iota", [128, fd], BF16, kind="ExternalInput")
    y_d = nc.dram_tensor("y", [n_words, 1024], F32, kind="ExternalOutput")

    # [n_iter, 128, w, 1024] views: iteration t covers words [rows_per_iter*t, ...)
    a_v = a_d[:].rearrange("(t s p) c -> t p s c", s=w, p=128)
    b_v = b_d[:].rearrange("(t s p) c -> t p s c", s=w, p=128)
    y_v = y_d[:].rearrange("(t s p) c -> t p s c", s=w, p=128)

    nseg = 4 * w  # one-hot segments per iteration tile

    with TileContext(nc) as tc:
        with (
            tc.tile_pool(name="cst", bufs=1) as cst,
            tc.tile_pool(name="ld", bufs=bufs_ld) as ld,
            tc.tile_pool(name="mul", bufs=bufs_mul) as mul,
            tc.tile_pool(name="idx", bufs=bufs_idx) as idxp,
            tc.tile_pool(name="sm", bufs=bufs_sm) as sm,
            tc.tile_pool(name="out", bufs=bufs_out) as outp,
        ):
            iota_sb = cst.tile([128, fd], BF16)
            nc.gpsimd.dma_start(iota_sb[:], iota_d[:])

            if mode == "dma2":
                # loads + one cheap DVE reader per tensor + store: DMA-bound probe
                for t in [t for _ in range(reps) for t in range(n_iter)]:
                    a_t = ld.tile([128, fd], BF16, tag="a")
                    nc.gpsimd.dma_start(
                        a_t[:].rearrange("p (s c) -> p s c", c=1024), a_v[t]
                    )
                    b_t = ld.tile([128, fd], BF16, tag="b")
                    nc.gpsimd.dma_start(
                        b_t[:].rearrange("p (s c) -> p s c", c=1024), b_v[t]
                    )
                    ma = mul.tile([128, fd], BF16, tag="m")
                    nc.vector.tensor_tensor(ma[:], a_t[:], iota_sb[:], OP.mult)
                    mb = outp.tile([128, fd], BF16, tag="o")
                    nc.vector.tensor_tensor(mb[:], b_t[:], iota_sb[:], OP.mult)
                    nc.gpsimd.dma_start(
                        y_v[t], mb[:].rearrange("p (s c) -> p s c", c=1024)
                    )

            if mode == "rw":
                # independent read + write streams (no data dependency)
                for t in [t for _ in range(reps) for t in range(n_iter)]:
                    a_t = ld.tile([128, fd], BF16, tag="a")
                    nc.gpsimd.dma_start(
                        a_t[:].rearrange("p (s c) -> p s c", c=1024), a_v[t]
                    )
                    b_t = ld.tile([128, fd], BF16, tag="b")
                    nc.gpsimd.dma_start(
                        b_t[:].rearrange("p (s c) -> p s c", c=1024), b_v[t]
                    )
                    zr = outp.tile([128, 16], BF16, tag="r")
                    nc.vector.tensor_scalar(
                        zr[:, 0:16], a_t[:, 0:16], 0.0, None, OP.mult
                    )
                    nc.vector.tensor_scalar(
                        zr[:, 0:1], b_t[:, 0:1], 0.0, None, OP.mult
                    )
                    zout = outp.tile([128, fd], BF16, tag="o")
                    nc.vector.tensor_scalar(zout[:], iota_sb[:], 0.0, None, OP.mult)
                    nc.gpsimd.dma_start(
                        y_v[t], zout[:].rearrange("p (s c) -> p s c", c=1024)
                    )

            if mode in ("dma", "dma_nocast", "ld", "st"):
                ldt = BF16 if mode != "dma_nocast" else F32
                for t in [t for _ in range(reps) for t in range(n_iter)]:
                    if mode != "st":
                        a_t = ld.tile([128, fd], ldt, tag="a")
                        nc.gpsimd.dma_start(
                            a_t[:].rearrange("p (s c) -> p s c", c=1024), a_v[t]
                        )
                        b_t = ld.tile([128, fd], ldt, tag="b")
                        nc.gpsimd.dma_start(
                            b_t[:].rearrange("p (s c) -> p s c", c=1024), b_v[t]
                        )
                    if mode == "ld":
                        zout = outp.tile([128, 16], BF16, tag="o")
                        nc.vector.tensor_scalar(
                            zout[:, 0:16], a_t[:, 0:16], 0.0, None, OP.mult
                        )
                        nc.vector.tensor_scalar(
                            zout[:, 0:1], b_t[:, 0:1], 0.0, None, OP.mult
                        )
                        continue
                    if mode == "st":
                        zout = outp.tile([128, fd], BF16, tag="o")
                        nc.vector.tensor_scalar(
                            zout[:], iota_sb[:], 0.0, None, OP.mult
                        )
                    else:
                        zout = outp.tile([128, fd], BF16, tag="o")
                        nc.vector.tensor_scalar(zout[:], a_t[:], 0.0, None, OP.mult)
                        nc.vector.tensor_scalar(
                            zout[:, 0:1], b_t[:, 0:1], 0.0, None, OP.mult
                        )
                    nc.gpsimd.dma_start(
                        y_v[t], zout[:].rearrange("p (s c) -> p s c", c=1024)
                    )

            if mode == "compute":
                a_c = cst.tile([128, fd], BF16)
                nc.gpsimd.dma_start(a_c[:].rearrange("p (s c) -> p s c", c=1024), a_v[0])
                b_c = cst.tile([128, fd], BF16)
                nc.gpsimd.dma_start(b_c[:].rearrange("p (s c) -> p s c", c=1024), b_v[0])

            pend_stores, gen_tail = [], []
            la_list, lb_list, st_list = [], [], []
            for t in (
                [t for _ in range(reps) for t in range(n_iter)]
                if mode in ("full", "compute", "serial")
                else []
            ):
                if mode == "compute":
                    a_t, b_t = a_c, b_c
                else:
                    a_t = ld.tile([128, fd], BF16, tag="a")
                    la_ins = nc.gpsimd.dma_start(
                        a_t[:].rearrange("p (s c) -> p s c", c=1024), a_v[t]
                    )
                    b_t = ld.tile([128, fd], BF16, tag="b")
                    lb_ins = nc.gpsimd.dma_start(
                        b_t[:].rearrange("p (s c) -> p s c", c=1024), b_v[t]
                    )
                    if mode == "serial":
                        la_list.append(la_ins)
                        lb_list.append(lb_ins)

                def extract_idx(src_t, tag):
                    # product with iota, then 2x-mode tree folds 256->16,
                    # then a short 1x reduce.  All partial sums exact in
                    # bf16: each 256-seg has exactly one nonzero (<=255).
                    ma = mul.tile([128, fd], BF16, tag="m")
                    nc.vector.tensor_tensor(ma[:], src_t[:], iota_sb[:], OP.mult)
                    cur = ma[:].rearrange("p (s c) -> p s c", c=256)
                    width = 256
                    while width > 16:
                        width //= 2
                        nxt_t = mul.tile([128, nseg * width], BF16, tag=f"f{width}")
                        nxt = nxt_t[:].rearrange("p (s c) -> p s c", c=width)
                        nc.vector.tensor_tensor(
                            nxt, cur[:, :, 0:width], cur[:, :, width : 2 * width],
                            OP.add,
                        )
                        cur = nxt
                    idx = idxp.tile([128, nseg], F32, tag=tag)
                    nc.vector.tensor_reduce(idx[:], cur, axis=AX.X, op=OP.add)
                    return idx

                idxa = extract_idx(a_t, "ia")
                idxb = extract_idx(b_t, "ib")

                # ripple-carry add over byte positions i=0..3 (i inner in col)
                def bslice(ap, i):
                    return ap.rearrange("p (s i) -> p i s", i=4)[:, i : i + 1, :]

                csum = idxp.tile([128, nseg], F32, tag="cs")
                carry = None
                for i in range(4):
                    t0 = sm.tile([128, w], F32, tag=f"t0{i}")
                    nc.vector.tensor_tensor(
                        t0[:].rearrange("p (i s) -> p i s", i=1),
                        bslice(idxa[:], i),
                        bslice(idxb[:], i),
                        OP.add,
                    )
                    if carry is not None:
                        nc.vector.tensor_tensor(t0[:], t0[:], carry[:], OP.add)
                    cnew = sm.tile([128, w], F32, tag=f"c{i}")
                    nc.vector.tensor_scalar(cnew[:], t0[:], 256.0, None, OP.is_ge)
                    nc.vector.scalar_tensor_tensor(
                        bslice(csum[:], i),
                        cnew[:].rearrange("p (i s) -> p i s", i=1),
                        -256.0,
                        t0[:].rearrange("p (i s) -> p i s", i=1),
                        OP.mult,
                        OP.add,
                    )
                    carry = cnew

                # xor with operand a (int32), back to bf16 for compares
                s_i = sm.tile([128, nseg], I32, tag="si")
                nc.vector.tensor_copy(s_i[:], csum[:])
                a_i = sm.tile([128, nseg], I32, tag="ai")
                nc.vector.tensor_copy(a_i[:], idxa[:])
                x_i = sm.tile([128, nseg], I32, tag="xi")
                nc.vector.tensor_tensor(x_i[:], s_i[:], a_i[:], OP.bitwise_xor)
                x_f = sm.tile([128, nseg], F32, tag="xf")
                nc.vector.tensor_copy(x_f[:], x_i[:])

                out_t = outp.tile([128, fd], BF16 if store == "gpsimd" else F32,
                                  tag="o")
                gen_ins = None
                for j in range(nseg):
                    gen_ins = nc.vector.tensor_scalar(
                        out_t[:, j * 256 : (j + 1) * 256],
                        iota_sb[:, 0:256],
                        x_f[:, j : j + 1],
                        None,
                        OP.is_equal,
                    )
                if mode != "compute":
                    eng = nc.gpsimd if store == "gpsimd" else nc.sync
                    st_ins = eng.dma_start(
                        y_v[t], out_t[:].rearrange("p (s c) -> p s c", c=1024)
                    )
                    if mode == "serial":
                        st_list.append(st_ins)
                    if batch_g and mode != "serial":
                        pend_stores.append(st_ins)
                        gen_tail.append(gen_ins)
                        k = len(gen_tail) - 1
                        if (k + 1) % batch_g == 0:
                            from concourse.tile import add_dep_helper
                            for si in pend_stores[:-1]:
                                add_dep_helper(si.ins, gen_tail[-1].ins, sync=True,
                                               reason="store batching")
                            pend_stores.clear(); gen_tail.clear()
            if mode == "compute":
                nc.gpsimd.dma_start(y_v[0], out_t[:].rearrange("p (s c) -> p s c", c=1024))

            if mode == "serial":
                # strict R/W phase alternation in groups of G iterations:
                # stores of group g wait on the group's last load; loads of
                # group g+1 wait on the last store of group g.
                from concourse.tile import add_dep_helper

                G = batch_g or 2
                n_total = len(st_list)
                for g0 in range(0, n_total, G):
                    grp = list(range(g0, min(g0 + G, n_total)))
                    last_load = lb_list[grp[-1]]
                    for t2 in grp:
                        add_dep_helper(
                            st_list[t2].ins, last_load.ins, sync=True,
                            reason="phaseRW",
                        )
                    nxt = g0 + G
                    if nxt < n_total:
                        add_dep_helper(
                            la_list[nxt].ins, st_list[grp[-1]].ins, sync=True,
                            reason="phaseWR",
                        )

    nc.compile()
    return nc


_CACHED = {}


def _get_kernel(n_words=BLOC, w=W):
    key = (n_words, w)
    if key not in _CACHED:
        _CACHED[key] = build_kernel(n_words, w)
    return _CACHED[key]


def _iota_tile(w=W):
    row = np.tile(np.arange(256, dtype=np.float32), 4 * w)
    return np.broadcast_to(row, (128, 1024 * w)).astype(ml_dtypes.bfloat16)


def kernel(**inputs):
    a = np.asarray(inputs["a_bytes"], dtype=np.float32).reshape(B, 1024)
    b = np.asarray(inputs["b_bytes"], dtype=np.float32).reshape(B, 1024)
    nc = _get_kernel()
    iota = _iota_tile()
    in_maps = [
        {
            "a": a[c * BLOC : (c + 1) * BLOC],
            "b": b[c * BLOC : (c + 1) * BLOC],
            "iota": iota,
        }
        for c in range(NCORES)
    ]
    res = bass_utils.run_bass_kernel_spmd(nc, in_maps, core_ids=list(range(NCORES)))
    out = np.concatenate([res.results[c]["y"] for c in range(NCORES)], axis=0)
    return out.reshape(B, 4, 256)



# revision 3
# speedup vs baseline: 1.6951x; 1.6951x over previous
"""Trainium2 Bass kernel for nn_C4ByteNibbleVM (v2).

The reference "soft VM" computes, per 32-bit word (4 bytes, one-hot f32
encoded), out = onehot(((a + b) mod 2^32) ^ a) bytewise with a ripple
carry.  With exact one-hot inputs every softmax in the reference is
saturated, so the reference output equals the exact integer result.
The kernel:
  1. extracts byte indices from the one-hot inputs,
  2. does the 4-byte ripple-carry add + xor in integer arithmetic,
  3. re-expands to one-hot via is_equal against an iota row.
Data parallel over the word dimension: 8192 words per core x 8 cores.

v2 changes vs v1:
  - inputs uploaded as bf16 (values 0/1 exact), output written as fp8
    (values 0/1 exact) -> HBM traffic per core drops 96 MB -> 40 MB.
  - index extraction: first fold each 256-wide one-hot segment to 128
    via lo-hi subtract (sign keeps bit7), multiply by a +-(r+1) iota,
    fold-tree to 16, reduce; decode |S|-1 + 128*[S<0].  This halves
    the DVE element traffic of extraction vs the v1 full-width
    multiply.
"""

import numpy as np
import ml_dtypes

import concourse.bacc as bacc
import concourse.mybir as mybir
from concourse.tile import TileContext
from concourse import bass_utils

B = 65536
NCORES = 8
BLOC = B // NCORES          # words per core
W = 8                       # 128-row chunks per iteration (1024 words)
ROWS_PER_ITER = 128 * W
NITER = BLOC // (128 * W)

F32 = mybir.dt.float32
BF16 = mybir.dt.bfloat16
FP8 = mybir.dt.float8e4
I32 = mybir.dt.int32
AX = mybir.AxisListType
OP = mybir.AluOpType


def build_kernel(n_words=BLOC, w=W, reps=1, store_dt=FP8):
    """Build the per-core Bass module. n_words must divide into 128*w tiles.

    reps>1 repeats the whole computation (same I/O) inside one NEFF so
    steady-state per-rep time can be measured by differencing wall times.
    """
    rows_per_iter = 128 * w
    n_iter = n_words // rows_per_iter
    fd = 1024 * w  # free dim of one iteration tile
    nseg = 4 * w   # one-hot segments per iteration tile

    nc = bacc.Bacc("TRN2", target_bir_lowering=False, debug=False)
    a_d = nc.dram_tensor("a", [n_words, 1024], BF16, kind="ExternalInput")
    b_d = nc.dram_tensor("b", [n_words, 1024], BF16, kind="ExternalInput")
    # iota_pm: [128, fd//2] values (r%128)+1 ; iota256: [128, 256] values 0..255
    iotap_d = nc.dram_tensor("iotap", [128, fd // 2], BF16, kind="ExternalInput")
    iota_d = nc.dram_tensor("iota", [128, 256], BF16, kind="ExternalInput")
    y_d = nc.dram_tensor("y", [n_words, 1024], store_dt, kind="ExternalOutput")

    # [n_iter, 128, w, 1024] views: iteration t covers words [rows_per_iter*t, ...)
    a_v = a_d[:].rearrange("(t s p) c -> t p s c", s=w, p=128)
    b_v = b_d[:].rearrange("(t s p) c -> t p s c", s=w, p=128)
    y_v = y_d[:].rearrange("(t s p) c -> t p s c", s=w, p=128)

    with TileContext(nc) as tc:
        with (
            tc.tile_pool(name="cst", bufs=1) as cst,
            tc.tile_pool(name="ld", bufs=2) as ld,
            tc.tile_pool(name="mul", bufs=2) as mul,
            tc.tile_pool(name="idx", bufs=2) as idxp,
            tc.tile_pool(name="sm", bufs=2) as sm,
            tc.tile_pool(name="out", bufs=2) as outp,
        ):
            iotap_sb = cst.tile([128, fd // 2], BF16)
            nc.gpsimd.dma_start(iotap_sb[:], iotap_d[:])
            iota_sb = cst.tile([128, 256], BF16)
            nc.gpsimd.dma_start(iota_sb[:], iota_d[:])

            for t in [t for _ in range(reps) for t in range(n_iter)]:
                a_t = ld.tile([128, fd], BF16, tag="a")
                nc.gpsimd.dma_start(
                    a_t[:].rearrange("p (s c) -> p s c", c=1024), a_v[t]
                )
                b_t = ld.tile([128, fd], BF16, tag="b")
                nc.gpsimd.dma_start(
                    b_t[:].rearrange("p (s c) -> p s c", c=1024), b_v[t]
                )

                def extract_idx(src_t, tag):
                    # signed fold 256->128: v1 = lo - hi (sign == bit7),
                    # multiply by (r+1), fold-tree 128->16, reduce, decode.
                    sv = src_t[:].rearrange("p (s h r) -> p s h r", h=2, r=128)
                    v1 = mul.tile([128, fd // 2], BF16, tag="v1")
                    v1v = v1[:].rearrange("p (s r) -> p s r", r=128)
                    nc.vector.tensor_tensor(
                        v1v, sv[:, :, 0:1, :].rearrange("p s h r -> p (s h) r"),
                        sv[:, :, 1:2, :].rearrange("p s h r -> p (s h) r"),
                        OP.subtract,
                    )
                    m = mul.tile([128, fd // 2], BF16, tag="m")
                    nc.vector.tensor_tensor(m[:], v1[:], iotap_sb[:], OP.mult)
                    cur = m[:].rearrange("p (s c) -> p s c", c=128)
                    width = 128
                    while width > 16:
                        width //= 2
                        nxt_t = mul.tile([128, nseg * width], BF16, tag=f"f{width}")
                        nxt = nxt_t[:].rearrange("p (s c) -> p s c", c=width)
                        nc.vector.tensor_tensor(
                            nxt, cur[:, :, 0:width], cur[:, :, width : 2 * width],
                            OP.add,
                        )
                        cur = nxt
                    S = idxp.tile([128, nseg], F32, tag=f"S{tag}")
                    nc.vector.tensor_reduce(S[:], cur, axis=AX.X, op=OP.add)
                    # S = +-(r+1); idx = |S| - 1 + 128*[S<0] = |S| + 127 - 128*[S>=0]
                    absS = idxp.tile([128, nseg], F32, tag=f"g{tag}")
                    nc.vector.scalar_tensor_tensor(
                        absS[:], S[:], -1.0, S[:], OP.mult, OP.max
                    )
                    gm = idxp.tile([128, nseg], F32, tag=f"t{tag}")
                    nc.vector.tensor_scalar(
                        gm[:], S[:], 0.0, -128.0, OP.is_ge, OP.mult
                    )
                    idx = idxp.tile([128, nseg], F32, tag=tag)
                    nc.vector.scalar_tensor_tensor(
                        idx[:], absS[:], 127.0, gm[:], OP.add, OP.add
                    )
                    return idx

                idxa = extract_idx(a_t, "ia")
                idxb = extract_idx(b_t, "ib")

                # ripple-carry add over byte positions i=0..3 (i inner in col)
                def bslice(ap, i):
                    return ap.rearrange("p (s i) -> p i s", i=4)[:, i : i + 1, :]

                csum = idxp.tile([128, nseg], F32, tag="cs")
                carry = None
                for i in range(4):
                    t0 = sm.tile([128, w], F32, tag=f"t0{i}")
                    nc.vector.tensor_tensor(
                        t0[:].rearrange("p (i s) -> p i s", i=1),
                        bslice(idxa[:], i),
                        bslice(idxb[:], i),
                        OP.add,
                    )
                    if carry is not None:
                        nc.vector.tensor_tensor(t0[:], t0[:], carry[:], OP.add)
                    cnew = sm.tile([128, w], F32, tag=f"c{i}")
                    nc.vector.tensor_scalar(cnew[:], t0[:], 256.0, None, OP.is_ge)
                    nc.vector.scalar_tensor_tensor(
                        bslice(csum[:], i),
                        cnew[:].rearrange("p (i s) -> p i s", i=1),
                        -256.0,
                        t0[:].rearrange("p (i s) -> p i s", i=1),
                        OP.mult,
                        OP.add,
                    )
                    carry = cnew

                # xor with operand a (int32), back to f32 for compares
                s_i = sm.tile([128, nseg], I32, tag="si")
                nc.vector.tensor_copy(s_i[:], csum[:])
                a_i = sm.tile([128, nseg], I32, tag="ai")
                nc.vector.tensor_copy(a_i[:], idxa[:])
                x_i = sm.tile([128, nseg], I32, tag="xi")
                nc.vector.tensor_tensor(x_i[:], s_i[:], a_i[:], OP.bitwise_xor)
                x_f = sm.tile([128, nseg], F32, tag="xf")
                nc.vector.tensor_copy(x_f[:], x_i[:])

                out_t = outp.tile([128, fd], BF16, tag="o")
                for j in range(nseg):
                    nc.vector.tensor_scalar(
                        out_t[:, j * 256 : (j + 1) * 256],
                        iota_sb[:, 0:256],
                        x_f[:, j : j + 1],
                        None,
                        OP.is_equal,
                    )
                nc.gpsimd.dma_start(
                    y_v[t], out_t[:].rearrange("p (s c) -> p s c", c=1024)
                )

    nc.compile()
    return nc


_CACHED = {}


def _get_kernel(n_words=BLOC, w=W):
    key = (n_words, w)
    if key not in _CACHED:
        _CACHED[key] = build_kernel(n_words, w)
    return _CACHED[key]


def _iotap_tile(w=W):
    row = np.tile(np.arange(1, 129, dtype=np.float32), 4 * w)
    return np.broadcast_to(row, (128, 512 * w)).astype(ml_dtypes.bfloat16)


def _iota256_tile():
    row = np.arange(256, dtype=np.float32)
    return np.broadcast_to(row, (128, 256)).astype(ml_dtypes.bfloat16)


def make_in_maps(a, b, w=W):
    """a, b: [B, 1024] float arrays -> per-core input dicts (bf16)."""
    a16 = np.asarray(a).astype(ml_dtypes.bfloat16)
    b16 = np.asarray(b).astype(ml_dtypes.bfloat16)
    iotap = _iotap_tile(w)
    iota = _iota256_tile()
    return [
        {
            "a": a16[c * BLOC : (c + 1) * BLOC],
            "b": b16[c * BLOC : (c + 1) * BLOC],
            "iotap": iotap,
            "iota": iota,
        }
        for c in range(NCORES)
    ]


def kernel(**inputs):
    a = np.asarray(inputs["a_bytes"], dtype=np.float32).reshape(B, 1024)
    b = np.asarray(inputs["b_bytes"], dtype=np.float32).reshape(B, 1024)
    nc = _get_kernel()
    in_maps = make_in_maps(a, b)
    res = bass_utils.run_bass_kernel_spmd(nc, in_maps, core_ids=list(range(NCORES)))
    out = np.concatenate(
        [res.results[c]["y"].astype(np.float32) for c in range(NCORES)], axis=0
    )
    return out.reshape(B, 4, 256)


# revision 4
# speedup vs baseline: 3.3762x; 1.9917x over previous
"""Trainium2 Bass kernel for nn_C4ByteNibbleVM (v3: PE-based extraction).

Inputs are uploaded transposed + fp8 (one-hot along rows): aT[c_row, word]
with c_row = byte*256 + c.  The TensorEngine extracts nibble indices:
for each (byte, half) plane the data slice [128 c-rows, 128 words] is the
STATIONARY operand and a tiny iota [128, 2] (nib_lo(c), nib_hi(c)) is the
MOVING operand; psum[word, 2] accumulates the two halves -> exact
(lo_nib, hi_nib) per word-byte.  DVE then does ripple-carry add + xor on
byte indices and one-hot generation via is_equal; store casts bf16->fp8.

Per core HBM traffic: 8 MB + 8 MB loads (fp8) + 8 MB store (fp8).
"""

import numpy as np
import ml_dtypes

import concourse.bacc as bacc
import concourse.mybir as mybir
from concourse.tile import TileContext
from concourse import bass_utils

B = 65536
NCORES = 8
BLOC = B // NCORES          # words per core
W = 8                       # 128-word chunks per iteration (1024 words)
ROWS_PER_ITER = 128 * W
NITER = BLOC // (128 * W)

F32 = mybir.dt.float32
BF16 = mybir.dt.bfloat16
FP8 = mybir.dt.float8e4
I32 = mybir.dt.int32
AX = mybir.AxisListType
OP = mybir.AluOpType


def build_kernel(n_words=BLOC, w=W, reps=1):
    rows_per_iter = 128 * w
    n_iter = n_words // rows_per_iter
    fd = 1024 * w  # one-hot free dim of one iteration (words*4*256 bytes)
    nseg = 4 * w   # (wchunk, byte) segments per iteration

    nc = bacc.Bacc("TRN2", target_bir_lowering=False, debug=False)
    # transposed one-hot inputs: row = byte*256 + (128*h + r), col = word
    a_d = nc.dram_tensor("a", [1024, n_words], FP8, kind="ExternalInput")
    b_d = nc.dram_tensor("b", [1024, n_words], FP8, kind="ExternalInput")
    # moving iota: [128, (h, 2)] cols (lo, hi) per half
    iotam_d = nc.dram_tensor("iotam", [128, 4], FP8, kind="ExternalInput")
    iota_d = nc.dram_tensor("iota", [128, 256], BF16, kind="ExternalInput")
    y_d = nc.dram_tensor("y", [n_words, 1024], FP8, kind="ExternalOutput")

    # input views: [plane(byte,h), 128 c-rows, word]
    a_v = a_d[:].rearrange("(pl r) w -> pl r w", r=128)
    b_v = b_d[:].rearrange("(pl r) w -> pl r w", r=128)
    y_v = y_d[:].rearrange("(t s p) c -> t p s c", s=w, p=128)

    with TileContext(nc) as tc:
        with (
            tc.tile_pool(name="cst", bufs=1) as cst,
            tc.tile_pool(name="ld", bufs=2) as ld,
            tc.tile_pool(name="ps", bufs=2, space="PSUM") as psp,
            tc.tile_pool(name="idx", bufs=2) as idxp,
            tc.tile_pool(name="sm", bufs=2) as sm,
            tc.tile_pool(name="out", bufs=2) as outp,
        ):
            iotam_sb = cst.tile([128, 4], FP8)
            nc.gpsimd.dma_start(iotam_sb[:], iotam_d[:])
            iota_sb = cst.tile([128, 256], BF16)
            nc.gpsimd.dma_start(iota_sb[:], iota_d[:])

            for t in [t for _ in range(reps) for t in range(n_iter)]:
                a_t = ld.tile([128, 8, rows_per_iter // 128 * 128], FP8, tag="a")
                nc.sync.dma_start(
                    a_t[:], a_v[:, :, t * rows_per_iter : (t + 1) * rows_per_iter]
                    .rearrange("pl r w -> r pl w")
                )
                b_t = ld.tile([128, 8, rows_per_iter], FP8, tag="b")
                nc.sync.dma_start(
                    b_t[:], b_v[:, :, t * rows_per_iter : (t + 1) * rows_per_iter]
                    .rearrange("pl r w -> r pl w")
                )

                # psum: [word_p, wchunk, tensor, byte, nib]
                ps = psp.tile([128, w, 2, 4, 2], F32, tag="ps")
                for k in range(w):
                    for ti, src in enumerate((a_t, b_t)):
                        for byte in range(4):
                            for h in range(2):
                                nc.tensor.matmul(
                                    ps[:, k, ti, byte, :],
                                    src[:, byte * 2 + h, k * 128 : (k + 1) * 128],
                                    iotam_sb[:, 2 * h : 2 * h + 2],
                                    start=(h == 0),
                                    stop=(h == 1),
                                )

                # evacuate psum once, then byte index = lo + 16*hi per tensor
                nib = idxp.tile([128, w, 2, 4, 2], F32, tag="nib")
                nc.vector.tensor_copy(nib[:], ps[:])
                idxa = idxp.tile([128, nseg], F32, tag="ia")
                nc.vector.scalar_tensor_tensor(
                    idxa[:].rearrange("p (k i) -> p k i", i=4),
                    nib[:, :, 0, :, 1], 16.0, nib[:, :, 0, :, 0],
                    OP.mult, OP.add,
                )
                idxb = idxp.tile([128, nseg], F32, tag="ib")
                nc.vector.scalar_tensor_tensor(
                    idxb[:].rearrange("p (k i) -> p k i", i=4),
                    nib[:, :, 1, :, 1], 16.0, nib[:, :, 1, :, 0],
                    OP.mult, OP.add,
                )

                # ripple-carry add over byte positions i=0..3 (i inner in col)
                def bslice(ap, i):
                    return ap.rearrange("p (s i) -> p i s", i=4)[:, i : i + 1, :]

                csum = idxp.tile([128, nseg], F32, tag="cs")
                carry = None
                for i in range(4):
                    t0 = sm.tile([128, w], F32, tag=f"t0{i}")
                    nc.vector.tensor_tensor(
                        t0[:].rearrange("p (i s) -> p i s", i=1),
                        bslice(idxa[:], i),
                        bslice(idxb[:], i),
                        OP.add,
                    )
                    if carry is not None:
                        nc.vector.tensor_tensor(t0[:], t0[:], carry[:], OP.add)
                    cnew = sm.tile([128, w], F32, tag=f"c{i}")
                    nc.vector.tensor_scalar(cnew[:], t0[:], 256.0, None, OP.is_ge)
                    nc.vector.scalar_tensor_tensor(
                        bslice(csum[:], i),
                        cnew[:].rearrange("p (i s) -> p i s", i=1),
                        -256.0,
                        t0[:].rearrange("p (i s) -> p i s", i=1),
                        OP.mult,
                        OP.add,
                    )
                    carry = cnew

                # xor with operand a (int32), back to f32 for compares
                s_i = sm.tile([128, nseg], I32, tag="si")
                nc.vector.tensor_copy(s_i[:], csum[:])
                a_i = sm.tile([128, nseg], I32, tag="ai")
                nc.vector.tensor_copy(a_i[:], idxa[:])
                x_i = sm.tile([128, nseg], I32, tag="xi")
                nc.vector.tensor_tensor(x_i[:], s_i[:], a_i[:], OP.bitwise_xor)
                x_f = sm.tile([128, nseg], F32, tag="xf")
                nc.vector.tensor_copy(x_f[:], x_i[:])

                out_t = outp.tile([128, fd], BF16, tag="o")
                for j in range(nseg):
                    nc.vector.tensor_scalar(
                        out_t[:, j * 256 : (j + 1) * 256],
                        iota_sb[:, 0:256],
                        x_f[:, j : j + 1],
                        None,
                        OP.is_equal,
                    )
                nc.gpsimd.dma_start(
                    y_v[t], out_t[:].rearrange("p (s c) -> p s c", c=1024)
                )

    nc.compile()
    return nc


_CACHED = {}


def _get_kernel(n_words=BLOC, w=W):
    key = (n_words, w)
    if key not in _CACHED:
        _CACHED[key] = build_kernel(n_words, w)
    return _CACHED[key]


def _iotam_tile():
    # cols [lo(h0), hi(h0), lo(h1), hi(h1)] for c = 128*h + r
    r = np.arange(128, dtype=np.float32)
    c0 = r
    c1 = 128 + r
    m = np.stack(
        [c0 % 16, c0 // 16, c1 % 16, c1 // 16], axis=1
    ).astype(np.float32)
    return m.astype(ml_dtypes.float8_e4m3)


def _iota256_tile():
    row = np.arange(256, dtype=np.float32)
    return np.broadcast_to(row, (128, 256)).astype(ml_dtypes.bfloat16)


def make_in_maps(a, b, w=W):
    """a, b: [B, 1024] float arrays -> per-core input dicts (transposed fp8)."""
    iotam = _iotam_tile()
    iota = _iota256_tile()
    maps = []
    for c in range(NCORES):
        asl = np.asarray(a[c * BLOC : (c + 1) * BLOC]).astype(ml_dtypes.float8_e4m3)
        bsl = np.asarray(b[c * BLOC : (c + 1) * BLOC]).astype(ml_dtypes.float8_e4m3)
        maps.append(
            {
                "a": np.ascontiguousarray(asl.T),
                "b": np.ascontiguousarray(bsl.T),
                "iotam": iotam,
                "iota": iota,
            }
        )
    return maps


def kernel(**inputs):
    a = np.asarray(inputs["a_bytes"], dtype=np.float32).reshape(B, 1024)
    b = np.asarray(inputs["b_bytes"], dtype=np.float32).reshape(B, 1024)
    nc = _get_kernel()
    in_maps = make_in_maps(a, b)
    res = bass_utils.run_bass_kernel_spmd(nc, in_maps, core_ids=list(range(NCORES)))
    out = np.concatenate(
        [res.results[c]["y"].astype(np.float32) for c in range(NCORES)], axis=0
    )
    return out.reshape(B, 4, 256)
